# revision 9
# baseline (speedup 1.0000x reference)
"""Linformer-style multihead attention on 8 Trainium2 NeuronCores.

Shapes (hardcoded): B=4, S=8192, D=512, H=8, DK=DV=64, PK=256.

Sharding: core c handles batch b=c//2, sequence half h=c%2 (4096 query rows).
The Linformer K/V projections contract over the FULL sequence, so each core
computes VP = We^T @ value[b] and VF = Wf^T @ value[b] over all 8192 rows
(redundant within a batch-pair, but avoids cross-core collectives).

Key algebra (reassociation): reference computes k = value@Wk then We^T@k.
We instead compute VP = We^T@value (8192-contraction) then kh = VP@Wk
(512-contraction). Biases fold in as rank-1 augmentation rows of the small
matmuls; the output bias bo is applied on the host after the gather.

On-chip pipeline is feature-major: query is transposed during DMA (bf16
x-bar transpose), phase B streams value/We/Wf in growing super-chunks on two
DMA queues (few descriptors, ~620ns trigger cost each) so the PE starts
within a few us and stays fed. The attention stage runs the q projection
per (s-tile, head-pair) interleaved with attention. Score matmuls for a
head pair run row-tiled (dk=64 halves of the PE array, concurrent); the AV
and softmax-denominator matmuls write col-tiled so the pair lands stacked
in one PSUM tile and the softmax normalize (reciprocal + multiply) is one
DVE op per pair instead of per head. Everything PE-side is bf16.
"""

import numpy as np
import ml_dtypes
from contextlib import ExitStack

import concourse.bass as bass
import concourse.bacc as bacc
import concourse.mybir as mybir
import concourse.tile as tile
from concourse import bass_utils
from concourse.masks import make_identity

B, S, D = 4, 8192, 512
H, DK, DV, PK = 8, 64, 64, 256
SH = S // 2  # per-core query rows
NCORES = 8
P = 128

F32 = mybir.dt.float32
BF16 = mybir.dt.bfloat16
AF = mybir.ActivationFunctionType
OP = mybir.AluOpType

_CACHE = {}

# phase-B super-chunk sizes in 512-row n-blocks (sum = 16); first ones small
# so the PE starts early, later ones large to amortize DMA trigger cost
SCHUNKS = [1, 1, 2, 4, 4, 4]


def _build_kernel():
    nc = bacc.Bacc(
        trn_type="TRN2",
        target_bir_lowering=False,
        debug=False,
        num_devices=NCORES,
    )

    q_t = nc.dram_tensor("q", [SH, D], BF16, kind="ExternalInput").ap()
    v_t = nc.dram_tensor("v", [S, D], BF16, kind="ExternalInput").ap()
    we_t = nc.dram_tensor("we", [S, PK], BF16, kind="ExternalInput").ap()
    wf_t = nc.dram_tensor("wf", [S, PK], BF16, kind="ExternalInput").ap()
    wq_t = nc.dram_tensor("wq", [D, D], BF16, kind="ExternalInput").ap()
    wk_t = nc.dram_tensor("wk", [D, D], BF16, kind="ExternalInput").ap()
    wv_t = nc.dram_tensor("wv", [D, D], BF16, kind="ExternalInput").ap()
    wo_t = nc.dram_tensor("wo", [D, D], BF16, kind="ExternalInput").ap()
    wkaug_t = nc.dram_tensor("wkaug", [2, D], BF16, kind="ExternalInput").ap()
    auge_t = nc.dram_tensor("auge", [2, PK], BF16, kind="ExternalInput").ap()
    wvaug_t = nc.dram_tensor("wvaug", [2, D], BF16, kind="ExternalInput").ap()
    augf_t = nc.dram_tensor("augf", [2, PK], BF16, kind="ExternalInput").ap()
    bq_t = nc.dram_tensor("bq", [D], F32, kind="ExternalInput").ap()
    out_t = nc.dram_tensor("out", [SH, D], F32, kind="ExternalOutput").ap()

    NT = SH // 512  # 8 s-tiles of 512

    with ExitStack() as ctx:
        tc = ctx.enter_context(tile.TileContext(nc))
        consts = ctx.enter_context(tc.tile_pool(name="consts", bufs=1))
        big = ctx.enter_context(tc.tile_pool(name="big", bufs=1))

        # ---- persistent activations ----
        qTraw = big.tile([P, 4, SH], BF16)   # query, feature-major
        khT = big.tile([P, 4, PK], BF16)     # [dk(2 heads/row-block), pair, pk]
        vh_sb = big.tile([P, 2, H, DV], BF16)  # [pk rows, chunk, head, dv]
        vpT = big.tile([P, 4, PK], BF16)
        vfT = big.tile([P, 4, PK], BF16)
        vp_sb = big.tile([P, 2, D], BF16)
        vf_sb = big.tile([P, 2, D], BF16)

        # ---- constants / weights on the sync queue (small, first) ----
        wq_sb = consts.tile([P, 4, D], BF16)
        nc.sync.dma_start(out=wq_sb, in_=wq_t.rearrange("(c p) e -> p c e", p=P))
        wk_sb = consts.tile([P, 4, D], BF16)
        nc.sync.dma_start(out=wk_sb, in_=wk_t.rearrange("(c p) e -> p c e", p=P))
        wv_sb = consts.tile([P, 4, D], BF16)
        nc.sync.dma_start(out=wv_sb, in_=wv_t.rearrange("(c p) e -> p c e", p=P))
        wo_sb = consts.tile([P, 4, D], BF16)
        nc.sync.dma_start(out=wo_sb, in_=wo_t.rearrange("(c p) e -> p c e", p=P))
        wkaug_sb = consts.tile([2, D], BF16)
        nc.sync.dma_start(out=wkaug_sb, in_=wkaug_t)
        auge_sb = consts.tile([2, PK], BF16)
        nc.sync.dma_start(out=auge_sb, in_=auge_t)
        wvaug_sb = consts.tile([2, D], BF16)
        nc.sync.dma_start(out=wvaug_sb, in_=wvaug_t)
        augf_sb = consts.tile([2, PK], BF16)
        nc.sync.dma_start(out=augf_sb, in_=augf_t)
        bq_sb = consts.tile([P, 4], F32)
        nc.sync.dma_start(out=bq_sb, in_=bq_t.rearrange("(c p) -> p c", p=P))
        ident_sb = consts.tile([P, P], BF16)
        make_identity(nc, ident_sb)
        ones64 = consts.tile([P, 64], BF16)
        nc.gpsimd.memset(ones64, 1.0)

        # ---- phase B: VP = We^T @ value, VF = Wf^T @ value (full S) ----
        # r=4 row-blocking: partition p of n-block n holds rows n*512+4p+r,
        # so each DMA piece is 4 contiguous rows (4KB for v, 2KB for We/Wf).
        # The contraction is just regrouped; We/v use the same grouping.
        v_r = v_t.rearrange("(n p r) d -> p n (r d)", p=P, r=4)
        we_r = we_t.rearrange("(n p r) k -> p n (r k)", p=P, r=4)
        wf_r = wf_t.rearrange("(n p r) k -> p n (r k)", p=P, r=4)
        NB = 16  # n-blocks of 512 rows
        with (
            tc.tile_pool(name="vstream", bufs=2) as vstream,
            tc.tile_pool(name="wstream", bufs=2) as wstream,
            tc.tile_pool(name="accp", bufs=4, space="PSUM") as accp,
        ):
            vp_ps = [accp.tile([P, D], F32, tag="acc", name=f"vp_ps{i}")
                     for i in range(2)]
            vf_ps = [accp.tile([P, D], F32, tag="acc", name=f"vf_ps{i}")
                     for i in range(2)]
            base = 0
            for si, sc in enumerate(SCHUNKS):
                csl = slice(base, base + sc)
                val_sb = vstream.tile([P, sc, 4, D], BF16, tag="val",
                                      name=f"val{si}", padded_shape=[P, 4, 4, D])
                nc.gpsimd.dma_start(
                    out=val_sb.rearrange("p n r d -> p n (r d)"),
                    in_=v_r[:, csl, :])
                we_sb = wstream.tile([P, sc, 4, PK], BF16, tag="we",
                                     name=f"we{si}", padded_shape=[P, 4, 4, PK])
                nc.sync.dma_start(
                    out=we_sb.rearrange("p n r k -> p n (r k)"),
                    in_=we_r[:, csl, :])
                wf_sb = wstream.tile([P, sc, 4, PK], BF16, tag="wf",
                                     name=f"wf{si}", padded_shape=[P, 4, 4, PK])
                nc.sync.dma_start(
                    out=wf_sb.rearrange("p n r k -> p n (r k)"),
                    in_=wf_r[:, csl, :])
                for i in range(sc):
                    for r in range(4):
                        k = (base + i) * 4 + r
                        first, last = (k == 0), (k == NB * 4 - 1)
                        for ps in range(2):
                            nc.tensor.matmul(
                                vp_ps[ps],
                                lhsT=we_sb[:, i, r, ps * P:(ps + 1) * P],
                                rhs=val_sb[:, i, r, :], start=first, stop=last)
                            nc.tensor.matmul(
                                vf_ps[ps],
                                lhsT=wf_sb[:, i, r, ps * P:(ps + 1) * P],
                                rhs=val_sb[:, i, r, :], start=first, stop=last)
                base += sc
            for ps in range(2):
                nc.vector.tensor_copy(out=vp_sb[:, ps, :], in_=vp_ps[ps])
                nc.vector.tensor_copy(out=vf_sb[:, ps, :], in_=vf_ps[ps])

        # query transpose during DMA (bf16 x-bar; queued on sync after the
        # phase-B We/Wf streams, done before attention starts)
        for dc in range(4):
            nc.sync.dma_start(
                out=qTraw[:, dc, :],
                in_=q_t[:, dc * P:(dc + 1) * P],
                transpose=True,
            )

        # transpose VP/VF to feature-major via PE (full-tile transpose)
        with tc.tile_pool(name="trp", bufs=4, space="PSUM") as trp:
            for ps in range(2):
                for eb in range(4):
                    tp = trp.tile([P, P], BF16, tag="tr", name=f"tp{ps}{eb}")
                    nc.tensor.transpose(
                        out=tp, in_=vp_sb[:, ps, eb * P:(eb + 1) * P],
                        identity=ident_sb)
                    nc.vector.tensor_copy(
                        out=vpT[:, eb, ps * P:(ps + 1) * P], in_=tp)
                    tf = trp.tile([P, P], BF16, tag="tr", name=f"tf{ps}{eb}")
                    nc.tensor.transpose(
                        out=tf, in_=vf_sb[:, ps, eb * P:(eb + 1) * P],
                        identity=ident_sb)
                    nc.vector.tensor_copy(
                        out=vfT[:, eb, ps * P:(ps + 1) * P], in_=tf)

        # khT[e', pk] = Wk^T @ VPT + rank-1 bias rows
        with tc.tile_pool(name="khp", bufs=2, space="PSUM") as khp:
            for pr in range(4):
                ps_t = khp.tile([P, PK], F32, tag="kh")
                for ec in range(4):
                    nc.tensor.matmul(
                        ps_t, lhsT=wk_sb[:, ec, pr * P:(pr + 1) * P],
                        rhs=vpT[:, ec, :], start=(ec == 0), stop=False)
                nc.tensor.matmul(
                    ps_t, lhsT=wkaug_sb[:, pr * P:(pr + 1) * P],
                    rhs=auge_sb, start=False, stop=True)
                nc.vector.tensor_copy(out=khT[:, pr, :], in_=ps_t)

        # vh[pk, dv] = VFT^T @ Wv + rank-1 bias rows (seq-major in pk)
        with tc.tile_pool(name="vhp", bufs=2, space="PSUM") as vhp:
            for ps in range(2):
                ps_t = vhp.tile([P, D], F32, tag="vh")
                for ec in range(4):
                    nc.tensor.matmul(
                        ps_t, lhsT=vfT[:, ec, ps * P:(ps + 1) * P],
                        rhs=wv_sb[:, ec, :], start=(ec == 0), stop=False)
                nc.tensor.matmul(
                    ps_t, lhsT=augf_sb[:, ps * P:(ps + 1) * P],
                    rhs=wvaug_sb, start=False, stop=True)
                nc.vector.tensor_copy(
                    out=vh_sb[:, ps, :, :],
                    in_=ps_t.rearrange("p (h v) -> p h v", h=H))

        # ---- attention: per s-tile, q-projection interleaved with
        #      pair-packed scores / softmax / AV / output projection ----
        out_r = out_t.rearrange("(t c p) d -> t p c d", c=4, p=P)
        with (
            tc.tile_pool(name="mm1", bufs=2, space="PSUM") as mm1,     # 2 banks
            tc.tile_pool(name="scp", bufs=2, space="PSUM") as scp,     # 2 banks
            tc.tile_pool(name="nzp", bufs=2, space="PSUM") as nzp,     # 4 banks
            tc.tile_pool(name="qstp", bufs=2) as qstp,
            tc.tile_pool(name="epool", bufs=6) as epool,
            tc.tile_pool(name="rzp", bufs=2) as rzp,
            tc.tile_pool(name="avp", bufs=2) as avp,
            tc.tile_pool(name="ostage", bufs=2) as ostage,
        ):
            for st in range(NT):
                ssl = slice(st * 512, (st + 1) * 512)
                qst = qstp.tile([P, 4, 512], BF16, tag="qst")
                av_sb = avp.tile([P, 4, 512], BF16, tag="av")
                # q projection for all four e-blocks first, so the vector
                # queue's qst copies run ahead of this s-tile's recip/mult
                # chain (in-order engine queues would otherwise serialize
                # pair j+1's scores behind pair j's softmax).
                for j in range(4):
                    qt = mm1.tile([P, 512], F32, tag="m1", name=f"qt{st}_{j}")
                    for dc in range(4):
                        nc.tensor.matmul(
                            qt, lhsT=wq_sb[:, dc, j * P:(j + 1) * P],
                            rhs=qTraw[:, dc, ssl],
                            start=(dc == 0), stop=(dc == 3))
                    nc.vector.tensor_scalar(
                        out=qst[:, j, :], in0=qt,
                        scalar1=bq_sb[:, j:j + 1], scalar2=None, op0=OP.add)
                for j in range(4):  # head pair (2j, 2j+1)
                    # scores for the pair: row-tiled (dk halves, concurrent),
                    # split by pk-half (ps) so exp/AV pipeline per chunk
                    es = []
                    for ps in range(2):
                        psl = slice(ps * P, (ps + 1) * P)
                        scA = scp.tile([P, 512], F32, tag="sc",
                                       name=f"scA{st}_{j}_{ps}")
                        scB = scp.tile([P, 512], F32, tag="sc",
                                       name=f"scB{st}_{j}_{ps}")
                        nc.tensor.matmul(
                            scA, lhsT=khT[0:64, j, psl],
                            rhs=qst[0:64, j, :], start=True, stop=True,
                            tile_position=(0, 0))
                        nc.tensor.matmul(
                            scB, lhsT=khT[64:P, j, psl],
                            rhs=qst[64:P, j, :], start=True, stop=True,
                            tile_position=(64, 0))
                        eA = epool.tile([P, 512], BF16, tag="e",
                                        name=f"eA{st}_{j}_{ps}")
                        eB = epool.tile([P, 512], BF16, tag="e",
                                        name=f"eB{st}_{j}_{ps}")
                        nc.scalar.activation(out=eA, in_=scA, func=AF.Exp)
                        nc.scalar.activation(out=eB, in_=scB, func=AF.Exp)
                        es.append((eA, eB))
                    # AV + denominator: pair stacked into one PSUM tile;
                    # nz[:,0,:] = numerator, nz[:,1,:] = Z (dup x64)
                    nz = nzp.tile([P, 2, 512], F32, tag="nz", name=f"nz{st}_{j}")
                    for c in range(2):
                        fl, ll = (c == 0), (c == 1)
                        eA, eB = es[c]
                        nc.tensor.matmul(
                            nz[0:64, 0, :], lhsT=vh_sb[:, c, 2 * j, :],
                            rhs=eA, start=fl, stop=ll, tile_position=(0, 0))
                        nc.tensor.matmul(
                            nz[64:P, 0, :], lhsT=vh_sb[:, c, 2 * j + 1, :],
                            rhs=eB, start=fl, stop=ll, tile_position=(0, 64))
                        nc.tensor.matmul(
                            nz[0:64, 1, :], lhsT=ones64[:, :],
                            rhs=eA, start=fl, stop=ll, tile_position=(0, 0))
                        nc.tensor.matmul(
                            nz[64:P, 1, :], lhsT=ones64[:, :],
                            rhs=eB, start=fl, stop=ll, tile_position=(0, 64))
                    rz = rzp.tile([P, 512], F32, tag="rz", name=f"rz{st}_{j}")
                    nc.vector.reciprocal_approx_fast(out=rz, in_=nz[:, 1, :])
                    nc.vector.tensor_tensor(
                        out=av_sb[:, j, :], in0=nz[:, 0, :], in1=rz,
                        op=OP.mult)
                # output projection for the s-tile (bo added on host)
                o_sb = ostage.tile([P, 4, D], F32, tag="ost")
                for sl in range(4):
                    o_t = mm1.tile([P, D], F32, tag="m1", name=f"ot{st}_{sl}")
                    for pr in range(4):
                        nc.tensor.matmul(
                            o_t, lhsT=av_sb[:, pr, sl * P:(sl + 1) * P],
                            rhs=wo_sb[:, pr, :], start=(pr == 0), stop=(pr == 3))
                    nc.vector.tensor_copy(out=o_sb[:, sl, :], in_=o_t)
                nc.sync.dma_start(out=out_r[st], in_=o_sb)

    nc.finalize()
    return nc


def _prep_inputs(inputs):
    bf = ml_dtypes.bfloat16
    f32 = np.float32
    q = np.ascontiguousarray(inputs["query"])
    v = np.ascontiguousarray(inputs["value"])
    We, Wf = np.asarray(inputs["We"]), np.asarray(inputs["Wf"])
    scale = np.float32(DK ** -0.5)
    ones = np.ones(D, f32)
    sWe = We.astype(f32).sum(0)
    sWf = Wf.astype(f32).sum(0)
    shared = {
        "we": We.astype(bf),
        "wf": Wf.astype(bf),
        "wq": (np.asarray(inputs["Wq"]) * scale).astype(bf),
        "wk": np.asarray(inputs["Wk"]).astype(bf),
        "wv": np.asarray(inputs["Wv"]).astype(bf),
        "wo": np.asarray(inputs["Wo"]).astype(bf),
        "wkaug": np.stack([np.asarray(inputs["bk"], f32), ones]).astype(bf),
        "auge": np.stack([sWe, np.asarray(inputs["be"], f32)]).astype(bf),
        "wvaug": np.stack([np.asarray(inputs["bv"], f32), ones]).astype(bf),
        "augf": np.stack([sWf, np.asarray(inputs["bf"], f32)]).astype(bf),
        "bq": (np.asarray(inputs["bq"]) * scale).astype(f32),
    }
    in_maps = []
    for c in range(NCORES):
        b, half = c // 2, c % 2
        m = dict(shared)
        m["q"] = np.ascontiguousarray(q[b, half * SH:(half + 1) * SH, :]).astype(bf)
        m["v"] = np.ascontiguousarray(v[b]).astype(bf)
        in_maps.append(m)
    return in_maps


def kernel(**inputs):
    if "nc" not in _CACHE:
        _CACHE["nc"] = _build_kernel()
    nc = _CACHE["nc"]
    in_maps = _prep_inputs(inputs)
    res = bass_utils.run_bass_kernel_spmd(nc, in_maps, core_ids=list(range(NCORES)))
    bo = np.asarray(inputs["bo"], np.float32)
    out = np.empty((B, S, D), np.float32)
    for c in range(NCORES):
        b, half = c // 2, c % 2
        out[b, half * SH:(half + 1) * SH, :] = res.results[c]["out"]
    out += bo
    return out
